# revision 51
# baseline (speedup 1.0000x reference)
"""GegenbauerKAN layer (alpha=1 -> Chebyshev-U basis) on 8 TRN2 NeuronCores.

Math: y[b,o] = sum_{i,d} C_d(tanh(x[b,i])) * W[i,o,d],  d=0..7,
where C_d are Gegenbauer(alpha=1) = Chebyshev-U polynomials.

Strategy (v14):
  - Data-parallel over batch: each of the 8 cores handles 2048 rows.
  - Transposed layout: the host feeds x^T (bf16) so the contraction
    index i lives on SBUF partitions with no on-device transposes.
  - On-device basis in fp32 via Chebyshev addition formulas:
        t  = tanh(x)            s4 = (2t)^2 = U2+1
        b3 = (s4-2)t = U3/2     q2 = (s4-1)^2 = U2^2
        b4 = q2-s4   = U4       b5 = (s4-2)b3 = (U5+2t)/2
        q3 = b3^2               b6 = 4q3-q2  = U6
        e4 = (s4-2)^2           b7 = (e4-2)b3 = U7/2
  - Matmul operands bf16 (basis rounded at the matmul boundary, weights
    rounded once on host): 216 ns/MM at N=512 (f32r was feed-limited at
    ~236), FWL fast weight loads, half the DMA bytes. Rel err ~5e-3 vs
    the 2e-2 gate.
  - k=0 (U_0 = 1) folded into a host-computed per-output bias added at
    PSUM eviction (no N=8 bias matmuls in the PE stream).
  - Asymmetric chunks [256,512,512,512,256]: the narrow first chunk
    makes the ramp-critical x/W DMAs small; the narrow last chunk makes
    the post-last-matmul eviction+DMA tail small.
  - Chunk 0 runs k-outer (consume basis/W in production order during
    the DMA ramp); later chunks run j-outer (one PSUM group open at a
    time). ~13 warmup matmuls on a memset tile bridge the DMA ramp so
    the PE HAM clock gate (1.2 -> 2.4 GHz) is released before real work.
  - DMA ring discipline (rings are in-order, ~100 GB/s each): the
    scalar ring carries ONLY the first x slice (bulk issues on it would
    stall the ACT FIFO); x on sync; W on gpsimd in (k,a) pieces;
    y alternates gpsimd/sync; the final y piece is split across two
    rings in parallel.
  - Weights basis change on host: V[:,:,k] = sum_d W[:,:,d] M[d,k]
    applied in fp64, rounded once to bf16.
"""

import numpy as np
import ml_dtypes

import concourse.bacc as bacc
import concourse.mybir as mybir
import concourse.tile as tile
from concourse.alu_op_type import AluOpType as ALU
from concourse.bass_utils import run_bass_kernel_spmd

F32 = mybir.dt.float32
BF16 = mybir.dt.bfloat16
AF = mybir.ActivationFunctionType

N_CORES = 8
B = 16384
I = 512
O = 512
DEG = 8  # degrees 0..7
B_LOC = B // N_CORES  # 2048 rows per core
CS = [256, 512, 512, 512, 256]  # chunk widths (batch columns)
OFF = [0, 256, 768, 1280, 1792]
N_CHUNKS = len(CS)
CMAX = 512
IT = I // 128  # 4 partition tiles of the input-feature dim
OT = O // 128  # 4 partition tiles of the output dim
N_WARMUP = 10  # HAM warmup matmuls (bridge the DMA ramp with no PE idle)


def _basis_matrix() -> np.ndarray:
    """M[d,k]: U_d = sum_k M[d,k] * phi_k for the on-device basis
    phi = [1, t, s4, b3, b4, b5, b6, b7]."""
    M = np.zeros((DEG, DEG))
    M[0, 0] = 1.0
    M[1, 1] = 2.0  # U1 = 2 t
    M[2, 0] = -1.0
    M[2, 2] = 1.0  # U2 = s4 - 1
    M[3, 3] = 2.0  # U3 = 2 b3
    M[4, 4] = 1.0  # U4 = b4
    M[5, 5] = 2.0
    M[5, 1] = -2.0  # U5 = 2 b5 - 2 t
    M[6, 6] = 1.0  # U6 = b6
    M[7, 7] = 2.0  # U7 = 2 b7
    return M


def _build_nc():
    nc = bacc.Bacc("TRN2", target_bir_lowering=False, debug=False)

    xt = nc.dram_tensor("xt", [I, B_LOC], BF16, kind="ExternalInput")
    # wv host layout: [p=128, k=7, a=IT, o=O] flattened to [128, 7*IT*O]
    wv = nc.dram_tensor("wv", [128, (DEG - 1) * IT * O], BF16, kind="ExternalInput")
    bias_in = nc.dram_tensor("bias_in", [128, OT], F32, kind="ExternalInput")
    yt = nc.dram_tensor("yt", [O, B_LOC], F32, kind="ExternalOutput")

    with tile.TileContext(nc) as tc:
        with (
            tc.tile_pool(name="wvp", bufs=1) as wvp,
            tc.tile_pool(name="sb", bufs=1) as sb,
            tc.tile_pool(name="xtp", bufs=1) as xtp,
            tc.tile_pool(name="outp", bufs=4) as outp,
            tc.tile_pool(name="ps", bufs=8, space="PSUM") as ps,
        ):
            # chunk 0's first x slice, first on the scalar ring: it gates
            # the whole pipeline (tanh -> cast -> first matmul)
            x_sb = [None] * N_CHUNKS
            x_sb[0] = xtp.tile([128, IT, CS[0]], BF16, tag="xt0", name="xt0")
            nc.scalar.dma_start(
                out=x_sb[0][:, 0, :], in_=xt[0:128, 0 : CS[0]]
            )

            # ---- PE warmup: lift the HAM clock gate during the DMA ramp ----
            junk = sb.tile([128, CMAX], BF16, tag="junk")
            nc.vector.memset(junk[:], 0.0)
            warm_ps = ps.tile([128, CMAX], F32, tag="acc")
            for i in range(N_WARMUP):
                nc.tensor.matmul(
                    warm_ps[:],
                    lhsT=junk[:, 0:128],
                    rhs=junk[:],
                    start=(i == 0),
                    stop=(i == N_WARMUP - 1),
                )

            # ---- input DMAs ----
            # rest of chunk 0's x a-slices on the sync ring
            for a in range(1, IT):
                nc.sync.dma_start(
                    out=x_sb[0][:, a, :],
                    in_=xt[a * 128 : (a + 1) * 128, 0 : CS[0]],
                )

            # weights on gpsimd, in consumption order; (k=1, a=0) goes in
            # four 32KB j-pieces so the very first matmuls are not gated
            # on a large transfer
            w_sb = wvp.tile([128, DEG - 1, IT, O], BF16, tag="wv")
            for jp in range(OT):
                nc.gpsimd.dma_start(
                    out=w_sb[:, 0, 0, jp * 128 : (jp + 1) * 128],
                    in_=wv[:, jp * 128 : (jp + 1) * 128],
                )
            for k in range(1, DEG):
                for a in range(IT):
                    if k == 1 and a == 0:
                        continue
                    off = (k - 1) * IT * O + a * O
                    nc.gpsimd.dma_start(
                        out=w_sb[:, k - 1, a, :],
                        in_=wv[:, off : off + O],
                    )

            # remaining x chunks (sync ring; chunk c is only needed once
            # the PE reaches chunk c-1, many microseconds later)
            for c in range(1, N_CHUNKS):
                x_sb[c] = xtp.tile(
                    [128, IT, CS[c]], BF16, tag=f"xt{c}", name=f"xt{c}"
                )
                nc.sync.dma_start(
                    out=x_sb[c][:],
                    in_=xt[:, OFF[c] : OFF[c] + CS[c]].rearrange(
                        "(a p) b -> p a b", p=128
                    ),
                )

            bias_sb = sb.tile([128, OT], F32, tag="bias")
            nc.sync.dma_start(out=bias_sb[:], in_=bias_in[:, :])

            neg1 = sb.tile([128, 1], F32, tag="neg1")
            nc.vector.memset(neg1[:], -1.0)
            neg2 = sb.tile([128, 1], F32, tag="neg2")
            nc.vector.memset(neg2[:], -2.0)

            def emit_dag(c, sliced):
                cs = CS[c]
                # fp32 intermediates (bufs=1, reused across chunks);
                # narrow chunks use the leading [:, :, :cs] slice
                t = sb.tile([128, IT, CMAX], F32, tag="t")
                s4 = sb.tile([128, IT, CMAX], F32, tag="s4")
                q2 = sb.tile([128, IT, CMAX], F32, tag="q2")
                q3 = sb.tile([128, IT, CMAX], F32, tag="q3")
                e4 = sb.tile([128, IT, CMAX], F32, tag="e4")
                # bf16 matmul-boundary basis (bufs=2: produced for chunk
                # c+1 while the PE still consumes chunk c)
                t_bf = sb.tile([128, IT, CMAX], BF16, tag="t_bf", bufs=2)
                s4_bf = sb.tile([128, IT, CMAX], BF16, tag="s4_bf", bufs=2)
                b3 = sb.tile([128, IT, CMAX], F32, tag="b3")
                b3_bf = sb.tile([128, IT, CMAX], BF16, tag="b3_bf", bufs=2)
                b4_bf = sb.tile([128, IT, CMAX], BF16, tag="b4_bf", bufs=2)
                b5_bf = sb.tile([128, IT, CMAX], BF16, tag="b5_bf", bufs=2)
                b6_bf = sb.tile([128, IT, CMAX], BF16, tag="b6_bf", bufs=2)
                b7_bf = sb.tile([128, IT, CMAX], BF16, tag="b7_bf", bufs=2)

                xs = x_sb[c]
                w = slice(0, cs)
                if sliced:
                    # slice-wise for the k<=3 path, interleaved so a late
                    # x slice does not stall the s4/b3 ladder of earlier
                    # slices in the strict-FIFO ACT/DVE queues
                    nc.scalar.activation(t[:, 0, w], xs[:, 0, :], AF.Tanh)
                    nc.vector.tensor_copy(t_bf[:, 0, w], t[:, 0, w])
                    nc.scalar.activation(t[:, 1, w], xs[:, 1, :], AF.Tanh)
                    nc.vector.tensor_copy(t_bf[:, 1, w], t[:, 1, w])
                    nc.scalar.activation(
                        s4[:, 0, w], t[:, 0, w], AF.Square, scale=2.0
                    )
                    nc.vector.tensor_copy(s4_bf[:, 0, w], s4[:, 0, w])
                    nc.scalar.activation(t[:, 2, w], xs[:, 2, :], AF.Tanh)
                    nc.vector.tensor_copy(t_bf[:, 2, w], t[:, 2, w])
                    nc.scalar.activation(
                        s4[:, 1, w], t[:, 1, w], AF.Square, scale=2.0
                    )
                    nc.vector.tensor_copy(s4_bf[:, 1, w], s4[:, 1, w])
                    nc.scalar.activation(t[:, 3, w], xs[:, 3, :], AF.Tanh)
                    nc.vector.tensor_copy(t_bf[:, 3, w], t[:, 3, w])
                    for a in (2, 3):
                        nc.scalar.activation(
                            s4[:, a, w], t[:, a, w], AF.Square, scale=2.0
                        )
                        nc.vector.tensor_copy(s4_bf[:, a, w], s4[:, a, w])
                    for a in range(IT):
                        nc.vector.scalar_tensor_tensor(
                            b3[:, a, w], s4[:, a, w], 2.0, t[:, a, w],
                            ALU.subtract, ALU.mult,
                        )
                        nc.vector.tensor_copy(b3_bf[:, a, w], b3[:, a, w])
                else:
                    nc.scalar.activation(
                        t[:, :, w], xs[:], AF.Tanh
                    )
                    nc.vector.tensor_copy(t_bf[:, :, w], t[:, :, w])
                    nc.scalar.activation(
                        s4[:, :, w], t[:, :, w], AF.Square, scale=2.0
                    )
                    nc.vector.tensor_copy(s4_bf[:, :, w], s4[:, :, w])
                    nc.vector.scalar_tensor_tensor(
                        b3[:, :, w], s4[:, :, w], 2.0, t[:, :, w],
                        ALU.subtract, ALU.mult,
                    )
                    nc.vector.tensor_copy(b3_bf[:, :, w], b3[:, :, w])

                nc.scalar.activation(
                    q2[:, :, w], s4[:, :, w], AF.Square, bias=neg1[:]
                )
                nc.vector.tensor_sub(b4_bf[:, :, w], q2[:, :, w], s4[:, :, w])
                nc.vector.scalar_tensor_tensor(
                    b5_bf[:, :, w], s4[:, :, w], 2.0, b3[:, :, w],
                    ALU.subtract, ALU.mult,
                )
                nc.scalar.activation(q3[:, :, w], b3[:, :, w], AF.Square)
                nc.vector.scalar_tensor_tensor(
                    b6_bf[:, :, w], q3[:, :, w], 4.0, q2[:, :, w],
                    ALU.mult, ALU.subtract,
                )
                nc.scalar.activation(
                    e4[:, :, w], s4[:, :, w], AF.Square, bias=neg2[:]
                )
                nc.vector.scalar_tensor_tensor(
                    b7_bf[:, :, w], e4[:, :, w], 2.0, b3[:, :, w],
                    ALU.subtract, ALU.mult,
                )
                return [t_bf, s4_bf, b3_bf, b4_bf, b5_bf, b6_bf, b7_bf]

            def emit_mms(c, basis):
                cs = CS[c]
                accs = [
                    ps.tile([128, CMAX], F32, tag="acc", name=f"acc{c}_{j}")
                    for j in range(OT)
                ]
                n_grp = (DEG - 1) * IT
                if c == 0:
                    # k-outer: consume basis tensors / W pieces in the
                    # order they are produced (ramp-friendly)
                    for k in range(1, DEG):
                        pk = basis[k - 1]
                        for a in range(IT):
                            for j in range(OT):
                                nc.tensor.matmul(
                                    accs[j][:, 0:cs],
                                    lhsT=w_sb[
                                        :, k - 1, a, j * 128 : (j + 1) * 128
                                    ],
                                    rhs=pk[:, a, 0:cs],
                                    start=(k == 1 and a == 0),
                                    stop=(k == DEG - 1 and a == IT - 1),
                                    skip_group_check=True,
                                )
                else:
                    # j-outer: one PSUM group open at a time (bank
                    # pressure), groups close staggered so eviction +
                    # y-DMA overlap the remaining matmuls
                    for j in range(OT):
                        idx = 0
                        for k in range(1, DEG):
                            for a in range(IT):
                                nc.tensor.matmul(
                                    accs[j][:, 0:cs],
                                    lhsT=w_sb[
                                        :, k - 1, a, j * 128 : (j + 1) * 128
                                    ],
                                    rhs=basis[k - 1][:, a, 0:cs],
                                    start=(idx == 0),
                                    stop=(idx == n_grp - 1),
                                    skip_group_check=True,
                                )
                                idx += 1
                return accs

            def emit_evictions(c, accs):
                cs = CS[c]
                bsl0 = OFF[c]
                g, sy, sc = nc.gpsimd, nc.sync, nc.scalar
                for j in range(OT):
                    o_sb = outp.tile([128, cs], F32, tag=f"out{cs}")
                    nc.scalar.activation(
                        o_sb[:], accs[j][:, 0:cs], AF.Identity,
                        bias=bias_sb[:, j : j + 1],
                    )
                    if c == N_CHUNKS - 1 and j == OT - 1:
                        # the very last piece: split the transfer across
                        # two otherwise-idle rings in parallel
                        h = cs // 2
                        for r, ring in enumerate([sy, sc]):
                            ring.dma_start(
                                out=yt[
                                    j * 128 : (j + 1) * 128,
                                    bsl0 + r * h : bsl0 + (r + 1) * h,
                                ],
                                in_=o_sb[:, r * h : (r + 1) * h],
                            )
                    else:
                        q = g if j % 2 == 0 else sy
                        q.dma_start(
                            out=yt[j * 128 : (j + 1) * 128, bsl0 : bsl0 + cs],
                            in_=o_sb[:],
                        )

            pending = None
            for c in range(N_CHUNKS):
                basis = emit_dag(c, sliced=(c == 0))
                if pending is not None:
                    # previous chunk's evictions AFTER this chunk's basis
                    # DAG so the strict-FIFO ACT queue prioritizes basis
                    emit_evictions(c - 1, pending)
                pending = emit_mms(c, basis)
            emit_evictions(N_CHUNKS - 1, pending)

    nc.compile()
    return nc


_NC_CACHE = None
_last_in_maps = None


def _get_nc():
    global _NC_CACHE
    if _NC_CACHE is None:
        _NC_CACHE = _build_nc()
    return _NC_CACHE


def kernel(x: np.ndarray, gegenbauer_coeffs: np.ndarray, **unused) -> np.ndarray:
    x = np.asarray(x, dtype=np.float32).reshape(B, I)
    coeffs = np.asarray(gegenbauer_coeffs, dtype=np.float32)
    xt_bf = np.ascontiguousarray(x.T).astype(ml_dtypes.bfloat16)  # [I, B]

    # Host prep: basis change (exact integers, applied in fp64), bias from
    # the k=0 block, and the [p, k, a, o] weight swizzle.
    M = _basis_matrix()
    v = np.einsum("iod,dk->kio", coeffs.astype(np.float64), M)  # [8, I, O]
    bias = v[0].sum(axis=0)  # [O]
    bias_dev = np.ascontiguousarray(
        bias.reshape(OT, 128).T.astype(np.float32)
    )  # [128, OT], bias_dev[p, j] = bias[j*128+p]
    # wv_host[p, k-1, a, o] = v[k, a*128+p, o]
    wv_host = np.ascontiguousarray(
        v[1:].reshape(DEG - 1, IT, 128, O).transpose(2, 0, 1, 3)
        .reshape(128, (DEG - 1) * IT * O)
        .astype(ml_dtypes.bfloat16)
    )

    in_maps = []
    for c in range(N_CORES):
        xt_c = np.ascontiguousarray(xt_bf[:, c * B_LOC : (c + 1) * B_LOC])
        in_maps.append({"xt": xt_c, "wv": wv_host, "bias_in": bias_dev})

    global _last_in_maps
    _last_in_maps = in_maps

    nc = _get_nc()
    try:
        res = run_bass_kernel_spmd(nc, in_maps, core_ids=list(range(N_CORES)))
    except Exception:
        # A previous crashed session can leave a core unrecoverable until
        # the runtime resets it; one retry clears it.
        res = run_bass_kernel_spmd(nc, in_maps, core_ids=list(range(N_CORES)))

    y = np.empty((B, O), dtype=np.float32)
    for c in range(N_CORES):
        y[c * B_LOC : (c + 1) * B_LOC, :] = res.results[c]["yt"].T
    return y


# revision 52
# speedup vs baseline: 1.0772x; 1.0772x over previous
"""GegenbauerKAN layer (alpha=1 -> Chebyshev-U basis) on 8 TRN2 NeuronCores.

Math: y[b,o] = sum_{i,d} C_d(tanh(x[b,i])) * W[i,o,d],  d=0..7,
where C_d are Gegenbauer(alpha=1) = Chebyshev-U polynomials.

Strategy (v2):
  - Data-parallel over batch: each of the 8 cores handles 2048 rows.
  - Transposed layout: the host feeds x^T slices so the contraction
    index i lives on SBUF partitions with no on-device transposes.
  - On-device basis: exact U_d values via Chebyshev addition formulas
    (U_{m+n} = U_m U_n - U_{m-1} U_{n-1}), computed in fp32:
        t  = tanh(x)            s4 = (2t)^2 = U2+1
        b3 = (s4-2)t = U3/2     q2 = (s4-1)^2 = U2^2
        b4 = q2-s4   = U4       b5 = (s4-2)b3 = (U5+2t)/2
        q3 = b3^2               b6 = 4q3-q2  = U6
        e4 = (s4-2)^2           b7 = (e4-2)b3 = U7/2
  - Matmul operands are bf16 (basis rounded at the matmul boundary,
    weights rounded once on host). Measured end-to-end rel err ~3e-3
    vs the 2e-2 gate. bf16 enables FWL fast weight loads and halves
    the SBUF feed bandwidth of the moving operand vs f32r.
  - k=0 (U_0 = 1) is folded into a per-output bias computed ON HOST
    (bias[o] = sum_i V[i,o,0]) and added at PSUM eviction; this
    removes 16 tiny N=8 matmuls from the PE stream.
  - k-outer matmul order per chunk: all 4 output-tile PSUM groups of a
    chunk stay open across k=1..7 so chunk 0 can start on k=1 as soon
    as tanh of the first x slice lands (sliced ramp), and so the W
    k-tiles are consumed in DMA arrival order.
  - A few warmup matmuls on a memset tile run during the DMA ramp to
    lift the PE HAM clock gate (1.2 -> 2.4 GHz) before real work.
  - Weights basis change on host: y = sum_k phi_k . V_k with
    V[:,:,k] = sum_d W[:,:,d] M[d,k], M the (exact, tiny) change of
    basis from {phi_k} to {U_d}; applied in fp64, rounded once.
"""

import numpy as np
import ml_dtypes

import concourse.bacc as bacc
import concourse.mybir as mybir
import concourse.tile as tile
from concourse.alu_op_type import AluOpType as ALU
from concourse.bass_utils import run_bass_kernel_spmd

F32 = mybir.dt.float32
BF16 = mybir.dt.bfloat16
AF = mybir.ActivationFunctionType

N_CORES = 8
B = 16384
I = 512
O = 512
DEG = 8  # degrees 0..7
B_LOC = B // N_CORES  # 2048 rows per core
CHUNK = 512  # b columns processed per pipeline stage
N_CHUNKS = B_LOC // CHUNK
IT = I // 128  # 4 partition tiles of the input-feature dim
OT = O // 128  # 4 partition tiles of the output dim
N_WARMUP = 13  # HAM warmup matmuls (bridge the DMA ramp with no PE idle)


def _basis_matrix() -> np.ndarray:
    """M[d,k]: U_d = sum_k M[d,k] * phi_k for the on-device basis
    phi = [1, t, s4, b3, b4, b5, b6, b7]."""
    M = np.zeros((DEG, DEG))
    M[0, 0] = 1.0
    M[1, 1] = 2.0  # U1 = 2 t
    M[2, 0] = -1.0
    M[2, 2] = 1.0  # U2 = s4 - 1
    M[3, 3] = 2.0  # U3 = 2 b3
    M[4, 4] = 1.0  # U4 = b4
    M[5, 5] = 2.0
    M[5, 1] = -2.0  # U5 = 2 b5 - 2 t
    M[6, 6] = 1.0  # U6 = b6
    M[7, 7] = 2.0  # U7 = 2 b7
    return M


def _build_nc():
    nc = bacc.Bacc("TRN2", target_bir_lowering=False, debug=False)

    xt = nc.dram_tensor("xt", [I, B_LOC], BF16, kind="ExternalInput")
    # wv host layout: [p=128, k=7, a=IT, o=O] flattened to [128, 7*IT*O]
    wv = nc.dram_tensor("wv", [128, (DEG - 1) * IT * O], BF16, kind="ExternalInput")
    bias_in = nc.dram_tensor("bias_in", [128, OT], F32, kind="ExternalInput")
    yt = nc.dram_tensor("yt", [O, B_LOC], F32, kind="ExternalOutput")

    with tile.TileContext(nc) as tc:
        with (
            tc.tile_pool(name="wvp", bufs=1) as wvp,
            tc.tile_pool(name="sb", bufs=1) as sb,
            tc.tile_pool(name="xtp", bufs=1) as xtp,
            tc.tile_pool(name="outp", bufs=4) as outp,
            tc.tile_pool(name="ps", bufs=6, space="PSUM") as ps,
            tc.tile_pool(name="psh", bufs=2, space="PSUM") as psh,
        ):
            # chunk 0's first x slice, first on the scalar ring: it gates
            # the whole pipeline (tanh -> cast -> first matmul)
            x_sb = [None] * N_CHUNKS
            x_sb[0] = xtp.tile([128, IT, CHUNK], BF16, tag="xt0", name="xt0")
            nc.scalar.dma_start(
                out=x_sb[0][:, 0, :], in_=xt[0:128, 0:CHUNK]
            )

            # ---- PE warmup: lift the HAM clock gate during the DMA ramp ----
            junk = sb.tile([128, CHUNK], BF16, tag="junk")
            nc.vector.memset(junk[:], 0.0)
            warm_ps = ps.tile([128, CHUNK], F32, tag="acc")
            for i in range(N_WARMUP):
                nc.tensor.matmul(
                    warm_ps[:],
                    lhsT=junk[:, 0:128],
                    rhs=junk[:],
                    start=(i == 0),
                    stop=(i == N_WARMUP - 1),
                )

            # ---- input DMAs ----
            # The scalar queue carries ONLY x0a0's descriptor: bulk-DMA
            # issues on it get backpressured and stall the ACT FIFO (tanh)
            # behind them. Everything else: x on sync, W on gpsimd, each
            # ring in consumption order.
            for a, q in [(1, nc.sync), (2, nc.scalar), (3, nc.sync)]:
                q.dma_start(
                    out=x_sb[0][:, a, :],
                    in_=xt[a * 128 : (a + 1) * 128, 0:CHUNK],
                )

            # per-(k,a) 128KB W slices: the k=1,a=0 matmuls only need the
            # first slice, so the PE is not gated on a full 512KB k-tile
            w_sb = wvp.tile([128, DEG - 1, IT, O], BF16, tag="wv")
            for k in range(1, DEG):
                for a in range(IT):
                    off = (k - 1) * IT * O + a * O
                    nc.gpsimd.dma_start(
                        out=w_sb[:, k - 1, a, :],
                        in_=wv[:, off : off + O],
                    )

            # remaining x chunks (sync queue; chunk c is only needed once
            # the PE reaches chunk c-1, many microseconds later)
            for c in range(1, N_CHUNKS):
                x_sb[c] = xtp.tile(
                    [128, IT, CHUNK], BF16, tag=f"xt{c}", name=f"xt{c}"
                )
                nc.sync.dma_start(
                    out=x_sb[c][:],
                    in_=xt[:, c * CHUNK : (c + 1) * CHUNK].rearrange(
                        "(a p) b -> p a b", p=128
                    ),
                )

            bias_sb = sb.tile([128, OT], F32, tag="bias")
            nc.sync.dma_start(out=bias_sb[:], in_=bias_in[:, :])

            neg1 = sb.tile([128, 1], F32, tag="neg1")
            nc.vector.memset(neg1[:], -1.0)
            neg2 = sb.tile([128, 1], F32, tag="neg2")
            nc.vector.memset(neg2[:], -2.0)

            flat = [128, IT * CHUNK]

            def emit_dag(c, sliced):
                # fp32 intermediates (bufs=1, reused across chunks)
                t = sb.tile([128, IT, CHUNK], F32, tag="t")
                s4 = sb.tile([128, IT, CHUNK], F32, tag="s4")
                q2 = sb.tile(flat, F32, tag="q2")
                q3 = sb.tile(flat, F32, tag="q3")
                e4 = sb.tile(flat, F32, tag="e4")
                # bf16 matmul-boundary basis (bufs=2: produced for chunk
                # c+1 while the PE still consumes chunk c)
                t_bf = sb.tile([128, IT, CHUNK], BF16, tag="t_bf", bufs=2)
                s4_bf = sb.tile([128, IT, CHUNK], BF16, tag="s4_bf", bufs=2)
                b3 = sb.tile([128, IT, CHUNK], F32, tag="b3")
                b3_bf = sb.tile([128, IT, CHUNK], BF16, tag="b3_bf", bufs=2)
                b4_bf = sb.tile(flat, BF16, tag="b4_bf", bufs=2)
                b5_bf = sb.tile(flat, BF16, tag="b5_bf", bufs=2)
                b6_bf = sb.tile(flat, BF16, tag="b6_bf", bufs=2)
                b7_bf = sb.tile(flat, BF16, tag="b7_bf", bufs=2)

                xs = x_sb[c]
                if sliced:
                    # slice-wise for the k<=3 path, interleaved so a late
                    # a=3 x slice does not stall the s4/b3 ladder of the
                    # earlier slices in the strict-FIFO ACT/DVE queues
                    nc.scalar.activation(t[:, 0, :], xs[:, 0, :], AF.Tanh)
                    nc.vector.tensor_copy(t_bf[:, 0, :], t[:, 0, :])
                    nc.scalar.activation(t[:, 1, :], xs[:, 1, :], AF.Tanh)
                    nc.vector.tensor_copy(t_bf[:, 1, :], t[:, 1, :])
                    nc.scalar.activation(
                        s4[:, 0, :], t[:, 0, :], AF.Square, scale=2.0
                    )
                    nc.vector.tensor_copy(s4_bf[:, 0, :], s4[:, 0, :])
                    nc.scalar.activation(t[:, 2, :], xs[:, 2, :], AF.Tanh)
                    nc.vector.tensor_copy(t_bf[:, 2, :], t[:, 2, :])
                    nc.scalar.activation(
                        s4[:, 1, :], t[:, 1, :], AF.Square, scale=2.0
                    )
                    nc.vector.tensor_copy(s4_bf[:, 1, :], s4[:, 1, :])
                    nc.scalar.activation(t[:, 3, :], xs[:, 3, :], AF.Tanh)
                    nc.vector.tensor_copy(t_bf[:, 3, :], t[:, 3, :])
                    for a in (2, 3):
                        nc.scalar.activation(
                            s4[:, a, :], t[:, a, :], AF.Square, scale=2.0
                        )
                        nc.vector.tensor_copy(s4_bf[:, a, :], s4[:, a, :])
                    for a in range(IT):
                        nc.vector.scalar_tensor_tensor(
                            b3[:, a, :], s4[:, a, :], 2.0, t[:, a, :],
                            ALU.subtract, ALU.mult,
                        )
                        nc.vector.tensor_copy(b3_bf[:, a, :], b3[:, a, :])
                else:
                    tf = t[:].rearrange("p a b -> p (a b)")
                    xf = xs[:].rearrange("p a b -> p (a b)")
                    s4f = s4[:].rearrange("p a b -> p (a b)")
                    b3f = b3[:].rearrange("p a b -> p (a b)")
                    nc.scalar.activation(tf, xf, AF.Tanh)
                    nc.vector.tensor_copy(
                        t_bf[:].rearrange("p a b -> p (a b)"), tf
                    )
                    nc.scalar.activation(s4f, tf, AF.Square, scale=2.0)
                    nc.vector.tensor_copy(
                        s4_bf[:].rearrange("p a b -> p (a b)"), s4f
                    )
                    nc.vector.scalar_tensor_tensor(
                        b3f, s4f, 2.0, tf, ALU.subtract, ALU.mult
                    )
                    nc.vector.tensor_copy(
                        b3_bf[:].rearrange("p a b -> p (a b)"), b3f
                    )

                s4f = s4[:].rearrange("p a b -> p (a b)")
                b3f = b3[:].rearrange("p a b -> p (a b)")
                nc.scalar.activation(q2[:], s4f, AF.Square, bias=neg1[:])
                nc.vector.tensor_sub(b4_bf[:], q2[:], s4f)
                nc.vector.scalar_tensor_tensor(
                    b5_bf[:], s4f, 2.0, b3f, ALU.subtract, ALU.mult
                )
                nc.scalar.activation(q3[:], b3f, AF.Square)
                nc.vector.scalar_tensor_tensor(
                    b6_bf[:], q3[:], 4.0, q2[:], ALU.mult, ALU.subtract
                )
                nc.scalar.activation(e4[:], s4f, AF.Square, bias=neg2[:])
                nc.vector.scalar_tensor_tensor(
                    b7_bf[:], e4[:], 2.0, b3f, ALU.subtract, ALU.mult
                )
                return [
                    (t_bf, True),
                    (s4_bf, True),
                    (b3_bf, True),
                    (b4_bf, False),
                    (b5_bf, False),
                    (b6_bf, False),
                    (b7_bf, False),
                ]

            def emit_mms(c, basis):
                n_full = OT if c < N_CHUNKS - 1 else 3
                accs = [
                    ps.tile([128, CHUNK], F32, tag="acc", name=f"acc{c}_{j}")
                    for j in range(n_full)
                ]

                def flat_basis(k):
                    pk, is3d = basis[k - 1]
                    return pk[:].rearrange("p a b -> p (a b)") if is3d else pk[:]

                if c == 0:
                    # k-outer: consume basis tensors / W k-tiles in the
                    # order they are produced (ramp-friendly)
                    for k in range(1, DEG):
                        pkf = flat_basis(k)
                        for a in range(IT):
                            for j in range(OT):
                                nc.tensor.matmul(
                                    accs[j][:],
                                    lhsT=w_sb[
                                        :, k - 1, a, j * 128 : (j + 1) * 128
                                    ],
                                    rhs=pkf[:, a * CHUNK : (a + 1) * CHUNK],
                                    start=(k == 1 and a == 0),
                                    stop=(k == DEG - 1 and a == IT - 1),
                                    skip_group_check=True,
                                )
                elif c < N_CHUNKS - 1:
                    # j-outer: one PSUM group open at a time (bank-pressure
                    # friendly: only 6 full banks exist); basis for this
                    # chunk was fully produced a chunk ahead
                    pkfs = [flat_basis(k) for k in range(1, DEG)]
                    n_grp = (DEG - 1) * IT
                    for j in range(OT):
                        idx = 0
                        for k in range(1, DEG):
                            for a in range(IT):
                                nc.tensor.matmul(
                                    accs[j][:],
                                    lhsT=w_sb[
                                        :, k - 1, a, j * 128 : (j + 1) * 128
                                    ],
                                    rhs=pkfs[k - 1][
                                        :, a * CHUNK : (a + 1) * CHUNK
                                    ],
                                    start=(idx == 0),
                                    stop=(idx == n_grp - 1),
                                    skip_group_check=True,
                                )
                                idx += 1
                else:
                    # last chunk j-outer: each PSUM group closes 28 MMs
                    # apart so eviction + y-DMA overlap the remaining MMs.
                    # j=3 is further split into two half-width groups so
                    # the final eviction is only 256 columns.
                    pkfs = [flat_basis(k) for k in range(1, DEG)]
                    n_grp = (DEG - 1) * IT
                    for j in range(3):
                        idx = 0
                        for k in range(1, DEG):
                            for a in range(IT):
                                nc.tensor.matmul(
                                    accs[j][:],
                                    lhsT=w_sb[
                                        :, k - 1, a, j * 128 : (j + 1) * 128
                                    ],
                                    rhs=pkfs[k - 1][
                                        :, a * CHUNK : (a + 1) * CHUNK
                                    ],
                                    start=(idx == 0),
                                    stop=(idx == n_grp - 1),
                                    skip_group_check=True,
                                )
                                idx += 1
                    Hh = CHUNK // 2
                    for h in range(2):
                        acc_h = psh.tile(
                            [128, Hh], F32, tag="acch", name=f"acch{h}"
                        )
                        idx = 0
                        for k in range(1, DEG):
                            for a in range(IT):
                                nc.tensor.matmul(
                                    acc_h[:],
                                    lhsT=w_sb[:, k - 1, a, 3 * 128 : 4 * 128],
                                    rhs=pkfs[k - 1][
                                        :,
                                        a * CHUNK + h * Hh : a * CHUNK
                                        + (h + 1) * Hh,
                                    ],
                                    start=(idx == 0),
                                    stop=(idx == n_grp - 1),
                                    skip_group_check=True,
                                )
                                idx += 1
                        accs.append(acc_h)
                return accs

            def emit_evictions(c, accs):
                bsl0 = c * CHUNK
                if c < N_CHUNKS - 1:
                    for j in range(OT):
                        o_sb = outp.tile([128, CHUNK], F32, tag="out")
                        nc.scalar.activation(
                            o_sb[:], accs[j][:], AF.Identity,
                            bias=bias_sb[:, j : j + 1],
                        )
                        # alternate rings: ~100GB/s per ring, and the
                        # final drain waits on the slowest ring's backlog
                        q = nc.gpsimd if j % 2 == 0 else nc.sync
                        q.dma_start(
                            out=yt[j * 128 : (j + 1) * 128, bsl0 : bsl0 + CHUNK],
                            in_=o_sb[:],
                        )
                else:
                    # final chunk: evict in half-tiles, each y transfer on
                    # its own ring, so the post-last-matmul tail is one
                    # 128KB transfer instead of 1MiB on one ring
                    g, sy, sc = nc.gpsimd, nc.sync, nc.scalar
                    # accs: j0..j2 full [128,512]; accs[3]/accs[4] are the
                    # two j=3 halves. The final half goes out as two
                    # parallel 128KB quarters on separate rings.
                    H = CHUNK // 2
                    Q = CHUNK // 4
                    plan = [
                        (0, accs[0], 0, H, [g]), (0, accs[0], H, H, [sy]),
                        (1, accs[1], 0, H, [g]), (1, accs[1], H, H, [sy]),
                        (2, accs[2], 0, H, [g]), (2, accs[2], H, H, [sy]),
                        (3, accs[3], 0, H, [g]),
                        # final piece: ONE eviction, transfer split into
                        # two parallel 64KB DMAs on separate rings
                        (3, accs[4], 0, H, [sy, sc]),
                    ]
                    for j, acc, off, width, rings in plan:
                        o_sb = outp.tile([128, width], F32, tag=f"o{width}")
                        nc.scalar.activation(
                            o_sb[:], acc[:, off : off + width],
                            AF.Identity, bias=bias_sb[:, j : j + 1],
                        )
                        ycol = bsl0 + (H if acc is accs[4] else 0) + off
                        w2 = width // len(rings)
                        for r, ring in enumerate(rings):
                            ring.dma_start(
                                out=yt[
                                    j * 128 : (j + 1) * 128,
                                    ycol + r * w2 : ycol + (r + 1) * w2,
                                ],
                                in_=o_sb[:, r * w2 : (r + 1) * w2],
                            )

            pending = None
            for c in range(N_CHUNKS):
                basis = emit_dag(c, sliced=(c == 0))
                if pending is not None:
                    # previous chunk's evictions AFTER this chunk's basis
                    # DAG so the strict-FIFO ACT queue prioritizes basis
                    emit_evictions(c - 1, pending)
                pending = emit_mms(c, basis)
            emit_evictions(N_CHUNKS - 1, pending)

    nc.compile()
    return nc


_NC_CACHE = None
_last_in_maps = None


def _get_nc():
    global _NC_CACHE
    if _NC_CACHE is None:
        _NC_CACHE = _build_nc()
    return _NC_CACHE


def kernel(x: np.ndarray, gegenbauer_coeffs: np.ndarray, **unused) -> np.ndarray:
    x = np.asarray(x, dtype=np.float32).reshape(B, I)
    coeffs = np.asarray(gegenbauer_coeffs, dtype=np.float32)
    xt_bf = np.ascontiguousarray(x.T).astype(ml_dtypes.bfloat16)  # [I, B]

    # Host prep: basis change (exact integers, applied in fp64), bias from
    # the k=0 block, and the [p, k, a, o] weight swizzle.
    M = _basis_matrix()
    v = np.einsum("iod,dk->kio", coeffs.astype(np.float64), M)  # [8, I, O]
    bias = v[0].sum(axis=0)  # [O]
    bias_dev = np.ascontiguousarray(
        bias.reshape(OT, 128).T.astype(np.float32)
    )  # [128, OT], bias_dev[p, j] = bias[j*128+p]
    # wv_host[p, k-1, a, o] = v[k, a*128+p, o]
    wv_host = np.ascontiguousarray(
        v[1:].reshape(DEG - 1, IT, 128, O).transpose(2, 0, 1, 3)
        .reshape(128, (DEG - 1) * IT * O)
        .astype(ml_dtypes.bfloat16)
    )
    in_maps = []
    for c in range(N_CORES):
        xt_c = np.ascontiguousarray(xt_bf[:, c * B_LOC : (c + 1) * B_LOC])
        in_maps.append({"xt": xt_c, "wv": wv_host, "bias_in": bias_dev})

    global _last_in_maps
    _last_in_maps = in_maps

    nc = _get_nc()
    try:
        res = run_bass_kernel_spmd(nc, in_maps, core_ids=list(range(N_CORES)))
    except Exception:
        # A previous crashed session can leave a core unrecoverable until
        # the runtime resets it; one retry clears it.
        res = run_bass_kernel_spmd(nc, in_maps, core_ids=list(range(N_CORES)))

    y = np.empty((B, O), dtype=np.float32)
    for c in range(N_CORES):
        y[c * B_LOC : (c + 1) * B_LOC, :] = res.results[c]["yt"].T
    return y
